# revision 13
# baseline (speedup 1.0000x reference)
"""Trainium2 Bass kernel for nn_CausalAnomalyDetector (B32 T512 D256 H8 L2).

Sharding: data-parallel over batch, 8 cores x 4 batch rows. Activations are
kept transposed [feature, token] in bf16; per-token scalars (LN stats,
softmax denominators) are built with ones-matmuls and broadcast back with
K=1 matmuls. Attention computes scores transposed [k, q] so the attnV
contraction streams probsT directly.
"""

import numpy as np
import ml_dtypes

import concourse.bass as bass
import concourse.mybir as mybir
import concourse.tile as tile
from concourse import bacc
from concourse.bass import IndirectOffsetOnAxis

BF16 = mybir.dt.bfloat16
F32 = mybir.dt.float32
I32 = mybir.dt.int32
AF = mybir.ActivationFunctionType
ALU = mybir.AluOpType

B, T, D, H, L, W = 32, 512, 256, 8, 2, 10
NQ = 10000
HD = D // H          # 32
FF = 4 * D           # 1024
NCORES = 8
BC = B // NCORES     # 4
NTOK = BC * T        # 2048
NSLAB = NTOK // 512  # 4
NTT = NTOK // 128    # 16
NW = NTOK // 128     # 16 (cols per partition in token-reshape)
SCALE = 1.0 / (HD ** 0.5)

nbf = ml_dtypes.bfloat16


def _bf(x):
    return np.ascontiguousarray(np.asarray(x, np.float32)).astype(nbf)


def _f32(x):
    return np.ascontiguousarray(np.asarray(x, np.float32))


_CACHED_NC = None


def _build_nc():
    nc = bacc.Bacc(None, target_bir_lowering=False)

    def din(name, shape, dt):
        return nc.declare_dram_parameter(name, list(shape), dt, isOutput=False)

    qm = din("qm", [NTOK, 1], I32)
    srow_bf = din("srow_bf", [1, NTOK], BF16)
    srows = din("srows", [BC, T], F32)
    scols = din("scols", [T, BC], BF16)
    qemb = din("qemb", [NQ + 1, D], BF16)
    posT = din("posT", [D, T], BF16)
    fus_wtop = din("fus_wtop", [D, D], BF16)
    fus_delta = din("fus_delta", [1, D], BF16)
    fus_bias = din("fus_bias", [D, 1], F32)
    wqk = din("wqk", [L, D, 2 * D], BF16)
    bqk = din("bqk", [L, 2 * D, 1], F32)
    wv = din("wv", [L, D, D], BF16)
    bv = din("bv", [L, 1, D], F32)
    wout = din("wout", [L, D, D], BF16)
    bout = din("bout", [L, D, 1], F32)
    lng = din("lng", [2 * L, 1, D], F32)
    lnb = din("lnb", [2 * L, 1, D], F32)
    wff1 = din("wff1", [L, D, FF], BF16)
    bff1 = din("bff1", [L, FF, 1], F32)
    wff2 = din("wff2", [L, FF, D], BF16)
    bff2 = din("bff2", [L, D, 1], F32)
    band10 = din("band10", [T, T], BF16)
    band5 = din("band5", [T, T], BF16)
    band9 = din("band9", [T, T], BF16)
    tril = din("tril", [128, 128], BF16)
    ident = din("ident", [128, 128], BF16)
    sw1dyn = din("sw1dyn", [4, 32], BF16)
    statk3_l = din("statk3_l", [3, 32], BF16)
    statk3_r = din("statk3_r", [3, T], BF16)
    sw2 = din("sw2", [32, 64], BF16)
    cw1 = din("cw1", [320, D], BF16)
    cb1 = din("cb1", [D, 1], F32)
    cw2 = din("cw2", [D, 1], BF16)
    cb2 = din("cb2", [1, 1], F32)
    run0 = din("run0", [1, T], F32)

    out_logits = nc.declare_dram_parameter("logits", [1, NTOK], F32, isOutput=True)

    with tile.TileContext(nc) as tc:
        with (
            tc.tile_pool(name="consts", bufs=1) as cp,
            tc.tile_pool(name="persist", bufs=1) as pp,
            tc.tile_pool(name="work", bufs=2) as wk,
            tc.tile_pool(name="probs", bufs=4) as prp,
            tc.tile_pool(name="gtp", bufs=2) as gtp,
            tc.tile_pool(name="psp", bufs=1, space="PSUM") as psp,
        ):
            # ---------------- resident constants / weights -----------------
            def load(shape, dt, src_ap, tag):
                t = cp.tile(list(shape), dt, tag=tag, name=tag)
                nc.sync.dma_start(out=t[:], in_=src_ap)
                return t

            sb_posT = [load([128, T], BF16, posT[128 * a:128 * (a + 1), :], f"posT{a}") for a in range(2)]
            sb_fwtop = [load([128, D], BF16, fus_wtop[128 * k:128 * (k + 1), :], f"fwtop{k}") for k in range(2)]
            sb_fdelta = load([1, D], BF16, fus_delta[:], "fdelta")
            sb_fbias = [load([128, 1], F32, fus_bias[128 * a:128 * (a + 1), :], f"fbias{a}") for a in range(2)]
            sb_wqk = [[load([128, 2 * D], BF16, wqk[l, 128 * k:128 * (k + 1), :], f"wqk{l}{k}") for k in range(2)] for l in range(L)]
            sb_bqk = [[load([128, 1], F32, bqk[l, 128 * f:128 * (f + 1), :], f"bqk{l}{f}") for f in range(4)] for l in range(L)]
            sb_wv = [[load([128, D], BF16, wv[l, 128 * k:128 * (k + 1), :], f"wv{l}{k}") for k in range(2)] for l in range(L)]
            sb_wout = [[load([128, D], BF16, wout[l, 128 * k:128 * (k + 1), :], f"wout{l}{k}") for k in range(2)] for l in range(L)]
            sb_bout = [[load([128, 1], F32, bout[l, 128 * f:128 * (f + 1), :], f"bout{l}{f}") for f in range(2)] for l in range(L)]
            sb_lng = [load([1, D], F32, lng[i], f"lng{i}") for i in range(2 * L)]
            sb_lnb = [load([1, D], F32, lnb[i], f"lnb{i}") for i in range(2 * L)]
            sb_wff1 = [[load([128, FF], BF16, wff1[l, 128 * k:128 * (k + 1), :], f"wff1{l}{k}") for k in range(2)] for l in range(L)]
            sb_bff1 = [[load([128, 1], F32, bff1[l, 128 * f:128 * (f + 1), :], f"bff1{l}{f}") for f in range(8)] for l in range(L)]
            sb_wff2 = [[load([128, D], BF16, wff2[l, 128 * k:128 * (k + 1), :], f"wff2{l}{k}") for k in range(8)] for l in range(L)]
            sb_bff2 = [[load([128, 1], F32, bff2[l, 128 * f:128 * (f + 1), :], f"bff2{l}{f}") for f in range(2)] for l in range(L)]
            sb_tril = load([128, 128], BF16, tril[:], "tril")
            sb_ident = load([128, 128], BF16, ident[:], "ident")
            sb_sw1dyn = load([4, 32], BF16, sw1dyn[:], "sw1dyn")
            sb_k3l = load([3, 32], BF16, statk3_l[:], "k3l")
            sb_k3r = load([3, T], BF16, statk3_r[:], "k3r")
            sb_sw2 = load([32, 64], BF16, sw2[:], "sw2")
            sb_cw1 = [load([128, D], BF16, cw1[128 * k:128 * (k + 1), :], f"cw1{k}") for k in range(2)]
            sb_cw1s = load([64, D], BF16, cw1[256:320, :], "cw1s")
            sb_cb1 = [load([128, 1], F32, cb1[128 * f:128 * (f + 1), :], f"cb1{f}") for f in range(2)]
            sb_cw2 = [load([128, 1], BF16, cw2[128 * k:128 * (k + 1), :], f"cw2{k}") for k in range(2)]
            sb_cb2 = load([1, 1], F32, cb2[:], "cb2")
            sb_srowbf = load([1, NTOK], BF16, srow_bf[:], "srowbf")
            sb_srows = load([BC, T], F32, srows[:], "srows")
            sb_scols = [load([128, BC], BF16, scols[128 * c:128 * (c + 1), :], f"scols{c}") for c in range(4)]
            sb_bv = [cp.tile([128, D], F32, tag=f"bv{l}", name=f"bv{l}") for l in range(L)]
            for l in range(L):
                nc.sync.dma_start(out=sb_bv[l][:], in_=bv[l].to_broadcast([128, D]))

            sb_ones_col = cp.tile([128, 32], BF16, tag="ones_col", name="ones_col")
            nc.vector.memset(sb_ones_col[:], 1.0)
            sb_ones_row = cp.tile([1, 512], F32, tag="ones_row", name="ones_row")
            nc.vector.memset(sb_ones_row[:], 1.0)
            sb_eps = cp.tile([128, 1], F32, tag="eps", name="eps")
            nc.vector.memset(sb_eps[:], 1e-5)

            # persistent activations
            xc = lambda: [pp.tile([128, NTOK], BF16, tag=f"xc{a}", name=f"xc{a}", bufs=2) for a in range(2)]
            x_t = xc()
            qkT = [pp.tile([128, NTOK], BF16, tag=f"qkT{f}", name=f"qkT{f}") for f in range(4)]
            v_n = [pp.tile([128, D], BF16, tag=f"v{t}", name=f"v{t}") for t in range(NTT)]
            semb = pp.tile([64, NTOK], BF16, tag="semb", name="semb")

            ln_args = (nc, wk, psp, sb_ones_col, sb_ones_row, sb_eps)

            # ---------------- stats path -----------------------------------
            d_ext = pp.tile([BC, T + 16], BF16, tag="dext", name="dext")
            nc.vector.memset(d_ext[:, 0:16], 1.0)
            dsub = pp.tile([BC, T - 1], F32, tag="dsub", name="dsub")
            nc.vector.tensor_tensor(out=dsub[:], in0=sb_srows[:, 1:T], in1=sb_srows[:, 0:T - 1], op=ALU.subtract)
            nc.vector.tensor_tensor(out=d_ext[:, 16:15 + T], in0=dsub[:], in1=dsub[:], op=ALU.mult)
            nc.vector.memset(d_ext[:, 15 + T:16 + T], 0.0)
            e_ext = pp.tile([BC, T + 16], BF16, tag="eext", name="eext")
            nc.vector.memset(e_ext[:, 0:16], 0.0)
            nc.vector.tensor_scalar(out=e_ext[:, 16:15 + T], in0=d_ext[:, 16:15 + T], scalar1=-1.0, scalar2=1.0, op0=ALU.mult, op1=ALU.add)
            nc.vector.memset(e_ext[:, 15:16], 0.0)
            nc.vector.memset(e_ext[:, 15 + T:16 + T], 0.0)

            runt = pp.tile([BC, T], F32, tag="runt", name="runt")
            nc.sync.dma_start(out=runt[:], in_=run0[:].to_broadcast([BC, T]))
            mxt = pp.tile([BC, T], F32, tag="mxt", name="mxt")
            nc.vector.tensor_copy(out=mxt[:], in_=runt[:])
            tmpr = pp.tile([BC, T], F32, tag="tmpr", name="tmpr")
            for j in range(1, W):
                nc.vector.tensor_tensor(out=tmpr[:], in0=runt[:], in1=e_ext[:, 6 + j: 6 + j + T], op=ALU.mult)
                nc.vector.tensor_scalar(out=runt[:], in0=tmpr[:], scalar1=1.0, scalar2=None, op0=ALU.add)
                nc.vector.tensor_tensor(out=mxt[:], in0=mxt[:], in1=runt[:], op=ALU.max)

            dcols = [wk.tile([128, BC], BF16, tag=f"dcols{c}", name=f"dcols{c}", bufs=1) for c in range(4)]
            for c in range(4):
                pst = psp.tile([128, 128], BF16, tag="mm", name="pst", space="PSUM", bufs=2)
                nc.tensor.transpose(out=pst[:, 0:BC], in_=d_ext[:, 16 + 128 * c: 16 + 128 * (c + 1)], identity=sb_ident[0:BC, 0:BC])
                nc.vector.tensor_copy(out=dcols[c][:], in_=pst[:, 0:BC])

            feats_all = pp.tile([BC, 4, T], BF16, tag="featsall", name="featsall")
            for fi, (bandd, colsrc) in enumerate(((band10, sb_scols), (band5, sb_scols), (band9, dcols))):
                psf = psp.tile([BC, T], F32, tag="mm", name="bandmm", space="PSUM", bufs=2)
                for c in range(4):
                    bt = wk.tile([128, T], BF16, tag="band", name="band", bufs=2)
                    nc.sync.dma_start(out=bt[:], in_=bandd[128 * c:128 * (c + 1), :])
                    nc.tensor.matmul(out=psf[:], lhsT=colsrc[c][:], rhs=bt[:], start=(c == 0), stop=(c == 3))
                nc.vector.tensor_copy(out=feats_all[:, fi, :], in_=psf[:])
            nc.vector.tensor_copy(out=feats_all[:, 3, :], in_=mxt[:])
            featsT = pp.tile([4, BC, T], BF16, tag="featsT", name="featsT")
            for fi in range(4):
                nc.sync.dma_start(out=featsT[fi:fi + 1, :, :], in_=feats_all[:, fi, :])
            for b in range(BC):
                ps1 = psp.tile([32, T], F32, tag="mm", name="mlp1", space="PSUM", bufs=2)
                nc.tensor.matmul(out=ps1[:], lhsT=sb_k3l[:], rhs=sb_k3r[:], start=True, stop=False)
                nc.tensor.matmul(out=ps1[:], lhsT=sb_sw1dyn[:], rhs=featsT[:, b, :], start=False, stop=True)
                r1 = wk.tile([32, T], BF16, tag="relu1", name="relu1")
                nc.scalar.activation(out=r1[:], in_=ps1[:], func=AF.Relu)
                ps2 = psp.tile([64, T], F32, tag="mm", name="mlp2", space="PSUM", bufs=2)
                nc.tensor.matmul(out=ps2[:], lhsT=sb_sw2[:], rhs=r1[:], start=True, stop=True)
                nc.vector.tensor_copy(out=semb[:, T * b: T * (b + 1)], in_=ps2[:])

            # ---------------- embedding gather + fusion --------------------
            qeT = [gtp.tile([128, NTOK], BF16, tag="gt0", name=f"qeT{a}") for a in range(2)]
            for tt in range(NTT):
                idx = wk.tile([128, 1], I32, tag="idx", name="idx")
                nc.sync.dma_start(out=idx[:], in_=qm[128 * tt: 128 * (tt + 1), :])
                qe = wk.tile([128, D], BF16, tag="qe", name="qe")
                nc.gpsimd.indirect_dma_start(
                    out=qe[:], out_offset=None, in_=qemb[:],
                    in_offset=IndirectOffsetOnAxis(ap=idx[:, :1], axis=0),
                )
                for a in range(2):
                    pst = psp.tile([128, 128], BF16, tag="mm", name="pst", space="PSUM", bufs=2)
                    nc.tensor.transpose(out=pst[:], in_=qe[:, 128 * a: 128 * (a + 1)], identity=sb_ident[:])
                    nc.vector.tensor_copy(out=qeT[a][:, 128 * tt: 128 * (tt + 1)], in_=pst[:])

            for s in range(NSLAB):
                sl = slice(512 * s, 512 * (s + 1))
                for a in range(2):
                    ps = psp.tile([128, 512], F32, tag="mm", name="mm", space="PSUM", bufs=2)
                    for k in range(2):
                        nc.tensor.matmul(out=ps[:], lhsT=sb_fwtop[k][:, 128 * a: 128 * (a + 1)], rhs=qeT[k][:, sl], start=(k == 0), stop=False)
                    nc.tensor.matmul(out=ps[:], lhsT=sb_fdelta[:, 128 * a: 128 * (a + 1)], rhs=sb_srowbf[:, sl], start=False, stop=True)
                    xa = wk.tile([128, 512], BF16, tag="fusev", name="fusev")
                    nc.scalar.activation(out=xa[:], in_=ps[:], func=AF.Relu, bias=sb_fbias[a][:, 0:1])
                    nc.vector.tensor_tensor(out=x_t[a][:, sl], in0=xa[:], in1=sb_posT[a][:], op=ALU.add)

            # ---------------- transformer layers ---------------------------
            for l in range(L):
                for s in range(NSLAB):
                    sl = slice(512 * s, 512 * (s + 1))
                    for f in range(4):
                        ps = psp.tile([128, 512], F32, tag="mm", name="mm", space="PSUM", bufs=2)
                        for k in range(2):
                            nc.tensor.matmul(out=ps[:], lhsT=sb_wqk[l][k][:, 128 * f: 128 * (f + 1)], rhs=x_t[k][:, sl], start=(k == 0), stop=(k == 1))
                        nc.vector.tensor_scalar(out=qkT[f][:, sl], in0=ps[:], scalar1=sb_bqk[l][f][:, 0:1], scalar2=None, op0=ALU.add)
                for tt in range(NTT):
                    ps = psp.tile([128, D], F32, tag="mm", name="mmv", space="PSUM", bufs=2)
                    for k in range(2):
                        nc.tensor.matmul(out=ps[:], lhsT=x_t[k][:, 128 * tt: 128 * (tt + 1)], rhs=sb_wv[l][k][:], start=(k == 0), stop=(k == 1))
                    nc.vector.tensor_tensor(out=v_n[tt][:], in0=ps[:], in1=sb_bv[l][:], op=ALU.add)

                x1 = [pp.tile([128, NTOK], BF16, tag=f"xr{f}", name=f"x1_{f}", bufs=1) for f in range(2)]
                for b in range(BC):
                    oTb = [wk.tile([128, 512], BF16, tag=f"oTb{g}", name=f"oTb{g}") for g in range(2)]
                    for g in range(2):
                        prs = []
                        for i in range(4):
                            qsl = slice(512 * b + 128 * i, 512 * (b + 1))
                            ncols = 512 - 128 * i
                            ksl = slice(512 * b + 128 * i, 512 * b + 128 * (i + 1))
                            sc = psp.tile([128, 4, 512], F32, tag="sc", name="sc", space="PSUM", bufs=1)
                            for h in range(4):
                                hp = slice(32 * h, 32 * (h + 1))
                                nc.tensor.matmul(out=sc[:, h, 0:ncols], lhsT=qkT[2 + g][hp, ksl], rhs=qkT[g][hp, qsl], start=True, stop=True, tile_position=(32 * h, 0))
                            pr = prp.tile([128, 4, 512], BF16, tag="pr", name="pr")
                            nc.scalar.activation(out=pr[:, :, 0:ncols], in_=sc[:, :, 0:ncols], func=AF.Exp, scale=SCALE)
                            trilap = sb_tril[:]
                            tril3 = bass.AP(tensor=trilap.tensor, offset=trilap.offset, ap=[trilap.ap[0], [0, 4], trilap.ap[1]])
                            nc.vector.tensor_tensor(out=pr[:, :, 0:128], in0=pr[:, :, 0:128], in1=tril3, op=ALU.mult)
                            prs.append(pr)
                        ot4 = psp.tile([128, 512], F32, tag="ot4", name="ot4", space="PSUM", bufs=1)
                        den4 = psp.tile([128, 512], F32, tag="den4", name="den4", space="PSUM", bufs=1)
                        for h in range(4):
                            hh = 4 * g + h
                            for i in range(4):
                                nc.tensor.matmul(out=ot4[32 * h: 32 * (h + 1), 128 * i:512], lhsT=v_n[4 * b + i][:, 32 * hh: 32 * (hh + 1)], rhs=prs[i][:, h, 0:512 - 128 * i], start=(i == 0), stop=(i == 3), tile_position=(0, 32 * h), skip_group_check=True)
                        for h in range(4):
                            for i in range(4):
                                nc.tensor.matmul(out=den4[32 * h: 32 * (h + 1), 128 * i:512], lhsT=sb_ones_col[:], rhs=prs[i][:, h, 0:512 - 128 * i], start=(i == 0), stop=(i == 3), tile_position=(0, 32 * h), skip_group_check=True)
                        rb = wk.tile([128, 512], F32, tag="rb", name="rb")
                        nc.vector.reciprocal_approx_fast(out=rb[:], in_=den4[:])
                        nc.vector.tensor_tensor(out=oTb[g][:], in0=ot4[:], in1=rb[:], op=ALU.mult)
                    sl = slice(512 * b, 512 * (b + 1))
                    for f in range(2):
                        ps = psp.tile([128, 512], F32, tag="mm", name="mm", space="PSUM", bufs=2)
                        for k in range(2):
                            nc.tensor.matmul(out=ps[:], lhsT=sb_wout[l][k][:, 128 * f: 128 * (f + 1)], rhs=oTb[k][:], start=(k == 0), stop=(k == 1))
                        nc.vector.scalar_tensor_tensor(out=x1[f][:, sl], in0=ps[:], scalar=sb_bout[l][f][:, 0:1], in1=x_t[f][:, sl], op0=ALU.add, op1=ALU.add)

                h1 = [pp.tile([128, NTOK], BF16, tag=f"h1_{f}", name=f"h1_{f}", bufs=1) for f in range(2)]
                _layernorm(ln_args, x1, h1, sb_lng[2 * l], sb_lnb[2 * l])

                x2 = [pp.tile([128, NTOK], BF16, tag=f"xr{f}", name=f"x2_{f}", bufs=1) for f in range(2)]
                for s in range(NSLAB):
                    sl = slice(512 * s, 512 * (s + 1))
                    gts = gtp.tile([128, 8, 512], BF16, tag="gt0", name="gts")
                    for f in range(8):
                        ps = psp.tile([128, 512], F32, tag="mm", name="mm", space="PSUM", bufs=2)
                        for k in range(2):
                            nc.tensor.matmul(out=ps[:], lhsT=sb_wff1[l][k][:, 128 * f: 128 * (f + 1)], rhs=h1[k][:, sl], start=(k == 0), stop=(k == 1))
                        nc.scalar.activation(out=gts[:, f, :], in_=ps[:], func=AF.Gelu, bias=sb_bff1[l][f][:, 0:1])
                    for f in range(2):
                        ps = psp.tile([128, 512], F32, tag="mm", name="mm", space="PSUM", bufs=2)
                        for k in range(8):
                            nc.tensor.matmul(out=ps[:], lhsT=sb_wff2[l][k][:, 128 * f: 128 * (f + 1)], rhs=gts[:, k, :], start=(k == 0), stop=(k == 7))
                        nc.vector.scalar_tensor_tensor(out=x2[f][:, sl], in0=ps[:], scalar=sb_bff2[l][f][:, 0:1], in1=h1[f][:, sl], op0=ALU.add, op1=ALU.add)

                xo = xc()
                _layernorm(ln_args, x2, xo, sb_lng[2 * l + 1], sb_lnb[2 * l + 1])
                x_t = xo

            # ---------------- classifier -----------------------------------
            uT = qkT  # reuse (dead after last attention)
            for s in range(NSLAB):
                sl = slice(512 * s, 512 * (s + 1))
                for f in range(2):
                    ps = psp.tile([128, 512], F32, tag="mm", name="mm", space="PSUM", bufs=2)
                    for k in range(2):
                        nc.tensor.matmul(out=ps[:], lhsT=sb_cw1[k][:, 128 * f: 128 * (f + 1)], rhs=x_t[k][:, sl], start=(k == 0), stop=False)
                    nc.tensor.matmul(out=ps[:], lhsT=sb_cw1s[:, 128 * f: 128 * (f + 1)], rhs=semb[:, sl], start=False, stop=True)
                    nc.scalar.activation(out=uT[f][:, sl], in_=ps[:], func=AF.Relu, bias=sb_cb1[f][:, 0:1])
            lg = psp.tile([1, NTOK], F32, tag="sc", name="lg", space="PSUM", bufs=1)
            for s in range(NSLAB):
                sl = slice(512 * s, 512 * (s + 1))
                for k in range(2):
                    nc.tensor.matmul(out=lg[0:1, sl], lhsT=sb_cw2[k][:], rhs=uT[k][:, sl], start=(k == 0), stop=(k == 1))
                lrow = wk.tile([1, 512], F32, tag="lrow", name="lrow")
                nc.vector.tensor_scalar(out=lrow[:], in0=lg[0:1, sl], scalar1=sb_cb2[0:1, 0:1], scalar2=None, op0=ALU.add)
                nc.sync.dma_start(out=out_logits[0:1, sl], in_=lrow[:])

    nc.compile()
    return nc


def _layernorm(ln_args, xin, xout, g_row, b_row):
    """xout = ((xin - mu) * rsqrt(var+eps)) * g + b over feature dim."""
    nc, wk, psp, ones_col, ones_row, eps = ln_args
    mu_sq = wk.tile([128, 32], F32, tag="ln_musq", name="ln_musq")
    for s in range(NSLAB):
        sl = slice(512 * s, 512 * (s + 1))
        sq = [wk.tile([128, 512], BF16, tag=f"ln_sq{a}", name=f"ln_sq{a}") for a in range(2)]
        for a in range(2):
            nc.vector.tensor_tensor(out=sq[a][:], in0=xin[a][:, sl], in1=xin[a][:, sl], op=ALU.mult)
        pmu = psp.tile([1, 512], F32, tag="mm", name="ln_pmu", space="PSUM", bufs=2)
        psq = psp.tile([1, 512], F32, tag="mm", name="ln_psq", space="PSUM", bufs=2)
        for a in range(2):
            nc.tensor.matmul(out=pmu[:], lhsT=ones_col[:, 0:1], rhs=xin[a][:, sl], start=(a == 0), stop=(a == 1))
        for a in range(2):
            nc.tensor.matmul(out=psq[:], lhsT=ones_col[:, 0:1], rhs=sq[a][:], start=(a == 0), stop=(a == 1))
        srow = wk.tile([1, 512], F32, tag="ln_srow", name="ln_srow")
        nc.vector.tensor_copy(out=srow[:], in_=pmu[:])
        nc.sync.dma_start(out=mu_sq[:, 4 * s: 4 * (s + 1)], in_=srow[:])
        srow2 = wk.tile([1, 512], F32, tag="ln_srow2", name="ln_srow2")
        nc.vector.tensor_copy(out=srow2[:], in_=psq[:])
        nc.sync.dma_start(out=mu_sq[:, 16 + 4 * s: 16 + 4 * (s + 1)], in_=srow2[:])
    mu2 = wk.tile([128, 16], F32, tag="ln_mu2", name="ln_mu2")
    nc.vector.scalar_tensor_tensor(out=mu2[:], in0=mu_sq[:, 0:16], scalar=1.0 / (D * D), in1=mu_sq[:, 0:16], op0=ALU.mult, op1=ALU.mult)
    var = wk.tile([128, 16], F32, tag="ln_var", name="ln_var")
    nc.vector.scalar_tensor_tensor(out=var[:], in0=mu_sq[:, 16:32], scalar=1.0 / D, in1=mu2[:], op0=ALU.mult, op1=ALU.subtract)
    std = wk.tile([128, 16], F32, tag="ln_std", name="ln_std")
    nc.scalar.activation(out=std[:], in_=var[:], func=AF.Sqrt, bias=eps[:, 0:1])
    rrv = wk.tile([128, 32], F32, tag="ln_rrv", name="ln_rrv")
    nc.vector.reciprocal(out=rrv[:, 0:16], in_=std[:])
    nc.vector.scalar_tensor_tensor(out=rrv[:, 16:32], in0=rrv[:, 0:16], scalar=-1.0 / D, in1=mu_sq[:, 0:16], op0=ALU.mult, op1=ALU.mult)
    for s in range(NSLAB):
        sl = slice(512 * s, 512 * (s + 1))
        rrow = wk.tile([1, 512], F32, tag="ln_rrow", name="ln_rrow")
        nrow = wk.tile([1, 512], F32, tag="ln_nrow", name="ln_nrow")
        nc.sync.dma_start(out=rrow[:], in_=rrv[:, 4 * s: 4 * (s + 1)])
        nc.sync.dma_start(out=nrow[:], in_=rrv[:, 16 + 4 * s: 16 + 4 * (s + 1)])
        for a in range(2):
            gsl = g_row[:, 128 * a: 128 * (a + 1)]
            bsl = b_row[:, 128 * a: 128 * (a + 1)]
            psA = psp.tile([128, 512], F32, tag="mm", name="ln_A", space="PSUM", bufs=2)
            nc.tensor.matmul(out=psA[:], lhsT=gsl, rhs=rrow[:], start=True, stop=True)
            psB = psp.tile([128, 512], F32, tag="mm", name="ln_B", space="PSUM", bufs=2)
            nc.tensor.matmul(out=psB[:], lhsT=gsl, rhs=nrow[:], start=True, stop=False)
            nc.tensor.matmul(out=psB[:], lhsT=bsl, rhs=ones_row[:], start=False, stop=True)
            tmp2 = wk.tile([128, 512], F32, tag="ln_t2", name="ln_t2")
            nc.vector.tensor_tensor(out=tmp2[:], in0=xin[a][:, sl], in1=psA[:], op=ALU.mult)
            nc.vector.tensor_tensor(out=xout[a][:, sl], in0=tmp2[:], in1=psB[:], op=ALU.add)


# ---------------------------------------------------------------------------
# host side
# ---------------------------------------------------------------------------

def _host_inputs(inputs):
    q = np.asarray(inputs["q"])
    s = np.asarray(inputs["s"])
    mask = s >= 0
    assert mask.all(), "kernel assumes all-valid mask (s >= 0)"
    qm = np.where(mask, q, 0).astype(np.int32)
    sm = np.where(mask, s, 0).astype(np.float32)

    f = {k: np.asarray(inputs[k], np.float32) for k in (
        "q_embed", "s_embed", "pos_embed", "fusion_w", "fusion_b", "in_proj_w",
        "in_proj_b", "out_proj_w", "out_proj_b", "ln1_g", "ln1_b", "ln2_g",
        "ln2_b", "ff1_w", "ff1_b", "ff2_w", "ff2_b", "sw1", "sb1", "sw2",
        "sb2", "cw1", "cb1", "cw2", "cb2")}

    w_top = f["fusion_w"][:D]
    w_bot = f["fusion_w"][D:]
    se_const = f["s_embed"][0] @ w_bot + f["fusion_b"]
    se_delta = (f["s_embed"][1] - f["s_embed"][0]) @ w_bot

    t_idx = np.arange(T, dtype=np.float32)
    valid = np.minimum(t_idx + 1, W)
    rcp_valid = 1.0 / (valid + 1e-6)
    rcp5 = 1.0 / (np.minimum(t_idx + 1, 5) + 1e-6)

    u = np.arange(T)[:, None]
    t = np.arange(T)[None, :]
    b10 = _bf(((u <= t) & (u > t - W)).astype(np.float32) * rcp_valid[None, :])
    b5 = _bf(((u <= t) & (u > t - 5)).astype(np.float32) * rcp5[None, :])
    # band9 consumes dcols rows: row r holds d[r+1] -> contributes to t where
    # (r+1) in (t-(W-2) .. t], i.e. sum_{u=t-8..t} d[u], u>=1
    b9 = _bf((((u + 1) <= t) & ((u + 1) > t - (W - 1))).astype(np.float32) * rcp_valid[None, :])

    kq = np.arange(128)
    trilm = _bf((kq[None, :] >= kq[:, None]).astype(np.float32))
    ident = _bf(np.eye(128, dtype=np.float32))

    rel = t_idx / max(T, 100)
    cov = valid / W
    sw1 = f["sw1"]
    sw1dyn = np.stack([sw1[0], sw1[1], sw1[2], sw1[3] / W])  # [4, 32]
    statk3_l = np.stack([sw1[4], sw1[5], f["sb1"]])
    statk3_r = np.stack([rel, cov, np.ones(T, np.float32)])

    cb1_eff = f["cb1"] + f["sb2"] @ f["cw1"][D:D + D // 4]
    run0 = (t_idx >= (W - 1)).astype(np.float32)

    shared = {
        "qemb": _bf(f["q_embed"]),
        "posT": _bf(f["pos_embed"][:T].T),
        "fus_wtop": _bf(w_top),
        "fus_delta": _bf(se_delta[None, :]),
        "fus_bias": _f32(se_const[:, None]),
        "wqk": _bf(f["in_proj_w"][:, :, :2 * D]),
        "bqk": _f32(f["in_proj_b"][:, :2 * D, None]),
        "wv": _bf(f["in_proj_w"][:, :, 2 * D:]),
        "bv": _f32(f["in_proj_b"][:, None, 2 * D:]),
        "wout": _bf(f["out_proj_w"]),
        "bout": _f32(f["out_proj_b"][:, :, None]),
        "lng": _f32(np.stack([f["ln1_g"][0], f["ln2_g"][0], f["ln1_g"][1], f["ln2_g"][1]])[:, None, :]),
        "lnb": _f32(np.stack([f["ln1_b"][0], f["ln2_b"][0], f["ln1_b"][1], f["ln2_b"][1]])[:, None, :]),
        "wff1": _bf(f["ff1_w"]),
        "bff1": _f32(f["ff1_b"][:, :, None]),
        "wff2": _bf(f["ff2_w"]),
        "bff2": _f32(f["ff2_b"][:, :, None]),
        "band10": b10, "band5": b5, "band9": b9,
        "tril": trilm, "ident": ident,
        "sw1dyn": _bf(sw1dyn),
        "statk3_l": _bf(statk3_l),
        "statk3_r": _bf(statk3_r),
        "sw2": _bf(f["sw2"]),
        "cw1": _bf(f["cw1"]),
        "cb1": _f32(cb1_eff[:, None]),
        "cw2": _bf(f["cw2"]),
        "cb2": _f32(np.asarray(f["cb2"]).reshape(1, 1)),
        "run0": _f32(run0[None, :]),
    }

    in_maps = []
    for c in range(NCORES):
        bs = slice(BC * c, BC * (c + 1))
        qc = qm[bs].reshape(NTOK)
        sc = sm[bs]
        m = dict(shared)
        m["qm"] = np.ascontiguousarray(qc[:, None])
        m["srow_bf"] = _bf(sc.reshape(1, NTOK))
        m["srows"] = _f32(sc)
        m["scols"] = _bf(sc.T)
        in_maps.append(m)
    return in_maps


def kernel(**inputs):
    global _CACHED_NC
    from concourse.bass_utils import run_bass_kernel_spmd
    if _CACHED_NC is None:
        _CACHED_NC = _build_nc()
    nc = _CACHED_NC
    in_maps = _host_inputs(inputs)
    res = run_bass_kernel_spmd(nc, in_maps, core_ids=list(range(NCORES)))
    out = np.concatenate([r["logits"].reshape(BC, T) for r in res.results], axis=0)
    return out.astype(np.float32)
